# revision 24
# baseline (speedup 1.0000x reference)
"""Trainium2 Bass kernel: 3x3 conv2d (stride 1, pad 1), NCHW.

x (32, 64, 112, 112) f32, weight (1, 128, 64, 3, 3) f32 -> out (32, 128, 112, 112) f32.

Strategy: data-parallel over batch across 8 cores (4 images/core).
x is host-padded to (114, 114) so each tap's shifted input window is a
constant free-dim offset into the flat [in_c=64, 114*114] SBUF image. Output
is produced in padded row-major (112 x 114) layout and sliced on the host.

Default variant (rt2i_fp16): the PE runs in row-tiled 64x128 mode — SBUF
partitions 0-63 hold one image, partitions 64-127 a second image, and the two
64-row PE tiles (T0/T8) execute their images' 9 K=64 tap matmuls
concurrently. That reaches the 4.5 matmul-slots-per-512-output-block math
floor for a 9-tap conv (an odd tap count can't be packed into K=128 pairs
without a wasted half-slot). fp16 operands/output (host upcasts): ~3.6e-4
rel err. Measured ~121.7us/pass steady-state vs ~184us for the original
6-matmul K=128 packing.
"""

import numpy as np

import concourse.bacc as bacc
import concourse.tile as tile
from concourse import mybir
from concourse.bass_utils import run_bass_kernel_spmd

# Problem constants (hardcoded per harness contract).
B, C, H, W = 32, 64, 112, 112
OC, KH, KW = 128, 3, 3
NCORES = 8
BPC = B // NCORES          # images per core
HP, WP = H + 2, W + 2      # host-padded input height/width (114)
XFLAT = HP * WP            # 12996 flat padded-input elements per channel
OFLAT = H * WP             # 12768 flat padded-output elements per channel
BLK = 512                  # matmul free-dim block (= 1 PSUM bank of fp32)
NBLK = (OFLAT + BLK - 1) // BLK  # 25 blocks (24 full + 1 of 480)
XBUF = XFLAT + 4           # SBUF image stride (matmul offsets read to XFLAT+1)
GS = 8                     # out-DMA grouping: 8 blocks per transfer
# Emit the output in fp16 (DVE casts during the PSUM->SBUF drain; the host
# upcasts to fp32). Halves the dominant out-DMA traffic for ~2^-11 extra
# rounding error. Applies to fp16 variants only.
OUT_FP16 = True

# matmul dtype: float32r streams fp32 through the PE at 1 cycle/row for
# free-dim >= 256 (vs 4 cycles/row for plain float32).
MM_DTYPE = mybir.dt.float32r

_cache = {}

# Variant switch (test harness flips this to isolate bottlenecks):
#   "rt2i_fp16" (default) - fp16, row-tiled 64x128 PE mode: partitions 0-63
#        hold image 2p, partitions 64-127 image 2p+1; PE tiles T0/T8 each run
#        the 9 K=64 tap matmuls of their own image CONCURRENTLY (9 MM slots
#        per 2 images per 512-block = the 4.5-slot math floor). No shifted
#        copies, no pair packing; input DMA halves (2 images per transfer).
#        Measured 121.7us/pass vs pack5 132.3, pack6 ~174.
#   "pack5_fp16" - fp16, 5 all-K=128 MMs per block: 3 tap-pair MMs
#        (taps (0,d)+(1,d) via the one-row-shifted copy on partitions 64-127)
#        + 1 pair MM (taps (2,0)+(2,1) via an on-device [img; img<<1] copy)
#        + 1 half-pair MM ([0; w(2,2)] at offset WP+2 on the row-shifted copy).
#   "pack6k128_fp16" - fp16 operands, 6 all-K=128 MMs per block:
#        3 tap-pair MMs + 3 "half-pair" MMs ([0; w(2,d)] at offset WP+d).
#        Measured ~2.9e-4 rel err; K=64 matmuls hit a much slower walrus
#        path, hence all-K=128.
#   "pack6"        - fp32r: 3 K=128 tap-pair MMs + 3 K=64 single MMs per block
#   "pack6_bf16"   - same structure as pack6, bf16 operands
#   "pack6k128_bf16" - as default but bf16
#   "wsplit9_bf16" - bf16 x on both partition halves; lhsT packs [w_hi; w_lo]
#                    per tap (w ~ w_hi + w_lo, near-fp32 weight precision);
#                    9 K=128 MMs per block
#   "wsplit9_ldw"  - wsplit9 tap-major with ldweights=False reuse (slower)
#   "mm_only" / "dma_only" - bottleneck-isolation probes
VARIANT = "rt2i_fp16"


def _build(repeat=1):
    """Build + compile the per-core Bass program (cached per process).

    repeat>1 runs the whole per-core conv `repeat` times back-to-back inside
    one NEFF (idempotent) — used by test.py to measure steady-state device
    time net of dispatch overhead.
    """
    key = ("nc", repeat, VARIANT)
    if key in _cache:
        return _cache[key]
    variant = VARIANT

    nc = bacc.Bacc("TRN2", target_bir_lowering=False, debug=False)
    if variant.endswith("fp16"):
        mm_dt = mybir.dt.float16
    elif variant.endswith("bf16") or variant in ("mm_only", "wsplit9_ldw"):
        mm_dt = mybir.dt.bfloat16
    else:
        mm_dt = MM_DTYPE
    if variant in ("wsplit9_bf16", "mm_only", "wsplit9_ldw", "rt9_fp16",
                   "rt2i_fp16"):
        nslot = 9
    elif variant in ("pack5_fp16", "pack5r_fp16"):
        nslot = 5
    else:
        nslot = 6
    assert variant in (
        "pack5_fp16",
        "pack5r_fp16",
        "rt9_fp16",
        "rt2i_fp16",
        "probe_samew_fp16",
        "probe_noldw_fp16",
        "probe_pure_fp16",
        "pack6",
        "pack6_bf16",
        "pack6k128_bf16",
        "pack6k128_fp16",
        "wsplit9_bf16",
        "wsplit9_ldw",
        "mm_only",
        "dma_only",
    ), variant
    # x arrives pre-doubled from the host: per image a [128, XBUF] block whose
    # partitions 0-63 hold the padded image (rows 0-113) and partitions 64-127
    # the same image shifted one row (pack6*) or repeated (wsplit9), so one
    # full-width DMA loads both copies.
    n_xblk = BPC // 2 if variant == "rt2i_fp16" else BPC
    x_ap = nc.dram_tensor(
        "x", [n_xblk, 2 * C, XBUF], mm_dt, kind="ExternalInput"
    ).ap()
    w_ap = nc.dram_tensor(
        "w", [2 * C, nslot * OC], mm_dt, kind="ExternalInput"
    ).ap()
    out_dt = (
        mybir.dt.float16
        if (OUT_FP16 and mm_dt == mybir.dt.float16)
        else mybir.dt.float32
    )
    out_ap = nc.dram_tensor(
        "out", [BPC, OC, H, WP], out_dt, kind="ExternalOutput"
    ).ap()

    with tile.TileContext(nc) as tc:
        from contextlib import ExitStack

        with ExitStack() as pools:
            xpool = pools.enter_context(tc.tile_pool(name="xpool", bufs=3))
            bpool = (
                pools.enter_context(tc.tile_pool(name="bpool", bufs=2))
                if variant in ("pack5_fp16", "pack5r_fp16")
                else None
            )
            wpool = pools.enter_context(tc.tile_pool(name="wpool", bufs=1))
            opool = pools.enter_context(tc.tile_pool(name="opool", bufs=4))
            pspool = pools.enter_context(
                tc.tile_pool(
                    name="psum",
                    bufs=4 if variant in ("rt9_fp16", "rt2i_fp16") else 8,
                    space="PSUM",
                )
            )
            # Weight slots: pack6* = 3 tap-pair slots (K=128: rows 0-63 =
            # tap (0,d), rows 64-127 = tap (1,d)) + 3 single slots (K=64:
            # tap (2,d)); wsplit9 = 9 taps x [w_hi; w_lo].
            wt = wpool.tile([2 * C, nslot * OC], mm_dt)
            nc.sync.dma_start(wt[:], w_ap[:])

            def conv_pass():
                if variant == "rt2i_fp16":
                    # Two images per SBUF tile: T0 (parts 0-63) = img 2p,
                    # T8 (parts 64-127) = img 2p+1. Each 64x128 PE tile runs
                    # all 9 K=64 taps of its own image; the two tiles stream
                    # concurrently. No shifted copy, no pair packing.
                    for p in range(BPC // 2):
                        xt = xpool.tile([2 * C, XBUF], mm_dt)
                        nc.sync.dma_start(xt[:], x_ap[p])
                        o_a = out_ap[2 * p].rearrange("o h w -> o (h w)")
                        o_b = out_ap[2 * p + 1].rearrange("o h w -> o (h w)")
                        ota = otb = None
                        for blk in range(NBLK):
                            j0 = blk * BLK
                            n = min(BLK, OFLAT - j0)
                            g = blk % GS
                            if g == 0:
                                ota = opool.tile([OC, GS * BLK], out_dt)
                                otb = opool.tile([OC, GS * BLK], out_dt)
                                g0 = j0
                            psA = pspool.tile([OC, BLK], mybir.dt.float32)
                            psB = pspool.tile([OC, BLK], mybir.dt.float32)
                            for t in range(KH * KW):
                                dh, dw = divmod(t, KW)
                                off = j0 + dh * WP + dw
                                nc.tensor.matmul(
                                    psA[:, :n],
                                    lhsT=wt[:C, t * OC : (t + 1) * OC],
                                    rhs=xt[:C, off : off + n],
                                    start=(t == 0),
                                    stop=(t == KH * KW - 1),
                                )
                                nc.tensor.matmul(
                                    psB[:, :n],
                                    lhsT=wt[C:, t * OC : (t + 1) * OC],
                                    rhs=xt[C:, off : off + n],
                                    start=(t == 0),
                                    stop=(t == KH * KW - 1),
                                )
                            nc.vector.tensor_copy(
                                ota[:, g * BLK : g * BLK + n], psA[:, :n]
                            )
                            nc.scalar.copy(
                                otb[:, g * BLK : g * BLK + n], psB[:, :n]
                            )
                            if g == GS - 1 or blk == NBLK - 1:
                                gn = j0 + n - g0
                                nc.sync.dma_start(
                                    o_a[:, g0 : g0 + gn], ota[:, :gn]
                                )
                                nc.sync.dma_start(
                                    o_b[:, g0 : g0 + gn], otb[:, :gn]
                                )
                    return
                for im in range(BPC):
                    # Partitions 0-63: padded image (rows 0-113).
                    # Partitions 64-127: same image shifted one row (+WP), so
                    # a K=128 matmul at offset j0+d contracts taps (0,d) and
                    # (1,d) simultaneously.
                    xt = xpool.tile([2 * C, XBUF], mm_dt)
                    nc.sync.dma_start(xt[:], x_ap[im])
                    o_im = out_ap[im].rearrange("o h w -> o (h w)")

                    xb = None
                    if variant in ("pack5_fp16", "pack5r_fp16"):
                        # xb = [img ; img<<1] over [2WP, XFLAT): the pair MM
                        # for taps (2,0)+(2,1) reads xb at offsets j0+2WP.
                        # Chunked so early blocks' MMs don't wait on the full
                        # copy; top half on the Act engine, bottom on DVE.
                        xb = bpool.tile([2 * C, XBUF], mm_dt)
                        nch = 4
                        lo, hi = 2 * WP, XFLAT
                        seg = (hi - lo + nch - 1) // nch
                        for s in range(nch):
                            a = lo + s * seg
                            b = min(lo + (s + 1) * seg, hi)
                            nc.scalar.copy(xb[:C, a:b], xt[:C, a:b])
                            nc.vector.tensor_copy(
                                xb[C:, a:b], xt[:C, a + 1 : b + 1]
                            )

                    if variant == "wsplit9_ldw":
                        # Tap-major over groups of GS blocks: one weight load
                        # per tap per group; the other GS-1 matmuls reuse the
                        # loaded weights (ldweights=False). PE instructions
                        # execute in FIFO program order, so the pairing holds.
                        for g0 in range(0, OFLAT, GS * BLK):
                            blks = [
                                (j0, min(BLK, OFLAT - j0))
                                for j0 in range(g0, min(g0 + GS * BLK, OFLAT), BLK)
                            ]
                            pss = [
                                pspool.tile(
                                    [OC, BLK],
                                    mybir.dt.float32,
                                    name=f"ps{bi}",
                                    tag="ps",
                                )
                                for bi in range(len(blks))
                            ]
                            for t in range(KH * KW):
                                dh, dw = divmod(t, KW)
                                for bi, (j0, n) in enumerate(blks):
                                    off = j0 + dh * WP + dw
                                    mm = nc.tensor.matmul(
                                        pss[bi][:, :n],
                                        lhsT=wt[:, t * OC : (t + 1) * OC],
                                        rhs=xt[:, off : off + n],
                                        start=(t == 0),
                                        stop=(t == KH * KW - 1),
                                    )
                                    if bi > 0:
                                        mm.ldweights = False
                            ot = opool.tile([OC, GS * BLK], out_dt)
                            for bi, (j0, n) in enumerate(blks):
                                nc.vector.tensor_copy(
                                    ot[:, bi * BLK : bi * BLK + n], pss[bi][:, :n]
                                )
                            gn = blks[-1][0] + blks[-1][1] - g0
                            nc.sync.dma_start(o_im[:, g0 : g0 + gn], ot[:, :gn])
                        continue

                    if variant == "pack5r_fp16":
                        # Tap-major over groups of RG blocks: one weight load
                        # per tap per group; the other RG-1 matmuls reuse the
                        # loaded weights (ldweights=False). RG=4 leaves 4 PSUM
                        # banks free so the next group's matmuls overlap this
                        # group's drains.
                        RG = 4
                        # (src, free-dim offset) per weight slot.
                        taps = [
                            (xt, 0),
                            (xt, 1),
                            (xt, 2),
                            (xb, 2 * WP),
                            (xt, WP + 2),
                        ]
                        ot = None
                        for gi, g0 in enumerate(range(0, OFLAT, RG * BLK)):
                            blks = [
                                (j0, min(BLK, OFLAT - j0))
                                for j0 in range(
                                    g0, min(g0 + RG * BLK, OFLAT), BLK
                                )
                            ]
                            pss = [
                                pspool.tile(
                                    [OC, BLK],
                                    mybir.dt.float32,
                                    name=f"ps{bi}",
                                    tag="ps",
                                )
                                for bi in range(len(blks))
                            ]
                            for t, (src, off) in enumerate(taps):
                                for bi, (j0, n) in enumerate(blks):
                                    mm = nc.tensor.matmul(
                                        pss[bi][:, :n],
                                        lhsT=wt[:, t * OC : (t + 1) * OC],
                                        rhs=src[:, j0 + off : j0 + off + n],
                                        start=(t == 0),
                                        stop=(t == len(taps) - 1),
                                    )
                                    if bi > 0:
                                        mm.ldweights = False
                            for bi, (j0, n) in enumerate(blks):
                                blk = gi * RG + bi
                                g = blk % GS
                                if g == 0:
                                    ot = opool.tile([OC, GS * BLK], out_dt)
                                    og0 = j0
                                nc.vector.tensor_copy(
                                    ot[:, g * BLK : g * BLK + n], pss[bi][:, :n]
                                )
                                if g == GS - 1 or blk == NBLK - 1:
                                    gn = j0 + n - og0
                                    nc.sync.dma_start(
                                        o_im[:, og0 : og0 + gn], ot[:, :gn]
                                    )
                        continue

                    ot = None
                    for blk in range(NBLK):
                        j0 = blk * BLK
                        n = min(BLK, OFLAT - j0)
                        g = blk % GS
                        if g == 0:
                            ot = opool.tile([OC, GS * BLK], out_dt)
                            g0 = j0
                        if variant == "rt9_fp16":
                            # Row-tiled 64x128 mode: tile T0 (SBUF parts 0-63,
                            # unshifted img) and T8 (parts 64-127, img<<WP)
                            # run K=64 matmuls CONCURRENTLY. 9 taps split 5/4
                            # per block (alternating to balance tiles across
                            # blocks). Per-block output = psA + psB (DVE add).
                            psA = pspool.tile([OC, BLK], mybir.dt.float32)
                            psB = pspool.tile([OC, BLK], mybir.dt.float32)
                            if blk % 2 == 0:
                                t0taps = [(0, 0), (0, 1), (0, 2), (2, 0), (2, 1)]
                                t8taps = [(1, 0), (1, 1), (1, 2), (2, 2)]
                            else:
                                t0taps = [(0, 0), (0, 1), (0, 2), (2, 0)]
                                t8taps = [(1, 0), (1, 1), (1, 2), (2, 1), (2, 2)]
                            # Interleave T0/T8 so both tiles stream without
                            # head-of-line blocking in the PE queue.
                            seq = []
                            for i in range(max(len(t0taps), len(t8taps))):
                                if i < len(t0taps):
                                    seq.append((0, t0taps[i]))
                                if i < len(t8taps):
                                    seq.append((1, t8taps[i]))
                            na = nb = 0
                            for which, (dh, dw) in seq:
                                t = 3 * dh + dw
                                if which == 0:
                                    off = j0 + dh * WP + dw
                                    na += 1
                                    nc.tensor.matmul(
                                        psA[:, :n],
                                        lhsT=wt[:C, t * OC : (t + 1) * OC],
                                        rhs=xt[:C, off : off + n],
                                        start=(na == 1),
                                        stop=(na == len(t0taps)),
                                    )
                                else:
                                    off = j0 + (dh - 1) * WP + dw
                                    nb += 1
                                    nc.tensor.matmul(
                                        psB[:, :n],
                                        lhsT=wt[C:, t * OC : (t + 1) * OC],
                                        rhs=xt[C:, off : off + n],
                                        start=(nb == 1),
                                        stop=(nb == len(t8taps)),
                                    )
                            nc.vector.tensor_add(
                                ot[:, g * BLK : g * BLK + n],
                                psA[:, :n],
                                psB[:, :n],
                            )
                        elif variant == "dma_only":
                            nc.vector.tensor_copy(
                                ot[:, g * BLK : g * BLK + n], xt[:OC, j0 : j0 + n]
                            )
                        elif variant.startswith("probe_"):
                            # Timing probes (WRONG numerics): 5 MMs per block
                            # all using weight slot 0. probe_noldw reuses the
                            # loaded weights on MMs 2-5 (ldweights=False) to
                            # isolate the serial LDWEIGHTS cost.
                            ps = pspool.tile([OC, BLK], mybir.dt.float32)
                            for d in range(5):
                                mm = nc.tensor.matmul(
                                    ps[:, :n],
                                    lhsT=wt[:, 0:OC],
                                    rhs=xt[:, j0 + d : j0 + d + n],
                                    start=(d == 0),
                                    stop=(d == 4),
                                )
                                if variant == "probe_noldw_fp16" and d > 0:
                                    mm.ldweights = False
                            if variant == "probe_pure_fp16":
                                continue  # no drain/out-DMA: pure PE pipeline
                            nc.vector.tensor_copy(
                                ot[:, g * BLK : g * BLK + n], ps[:, :n]
                            )
                        elif variant == "pack5_fp16":
                            ps = pspool.tile([OC, BLK], mybir.dt.float32)
                            for d in range(3):
                                nc.tensor.matmul(
                                    ps[:, :n],
                                    lhsT=wt[:, d * OC : (d + 1) * OC],
                                    rhs=xt[:, j0 + d : j0 + d + n],
                                    start=(d == 0),
                                    stop=False,
                                )
                            # Pair [w(2,0); w(2,1)]: xb top = img@(j0+2WP)
                            # (tap (2,0)), bottom = img@(j0+2WP+1) (tap (2,1)).
                            nc.tensor.matmul(
                                ps[:, :n],
                                lhsT=wt[:, 3 * OC : 4 * OC],
                                rhs=xb[:, j0 + 2 * WP : j0 + 2 * WP + n],
                                start=False,
                                stop=False,
                            )
                            # [0; w(2,2)] on the row-shifted copy: bottom =
                            # img@(j0+2WP+2) (tap (2,2)); zero top half.
                            nc.tensor.matmul(
                                ps[:, :n],
                                lhsT=wt[:, 4 * OC : 5 * OC],
                                rhs=xt[:, j0 + WP + 2 : j0 + WP + 2 + n],
                                start=False,
                                stop=True,
                            )
                            nc.vector.tensor_copy(
                                ot[:, g * BLK : g * BLK + n], ps[:, :n]
                            )
                        elif variant in ("wsplit9_bf16", "mm_only"):
                            ps = pspool.tile([OC, BLK], mybir.dt.float32)
                            for t in range(KH * KW):
                                dh, dw = divmod(t, KW)
                                off = j0 + dh * WP + dw
                                nc.tensor.matmul(
                                    ps[:, :n],
                                    lhsT=wt[:, t * OC : (t + 1) * OC],
                                    rhs=xt[:, off : off + n],
                                    start=(t == 0),
                                    stop=(t == KH * KW - 1),
                                )
                            if variant == "mm_only":
                                continue
                            nc.vector.tensor_copy(
                                ot[:, g * BLK : g * BLK + n], ps[:, :n]
                            )
                        else:
                            ps = pspool.tile([OC, BLK], mybir.dt.float32)
                            k128 = variant.startswith("pack6k128")
                            for d in range(3):
                                nc.tensor.matmul(
                                    ps[:, :n],
                                    lhsT=wt[:, d * OC : (d + 1) * OC],
                                    rhs=xt[:, j0 + d : j0 + d + n],
                                    start=(d == 0),
                                    stop=False,
                                )
                            for d in range(3):
                                if k128:
                                    # Slot 3+d = [0; w(2,d)]: upper half (copy
                                    # B, +WP shift) contributes tap (2,d) at
                                    # offset WP+d; lower half is zeroed.
                                    nc.tensor.matmul(
                                        ps[:, :n],
                                        lhsT=wt[:, (3 + d) * OC : (4 + d) * OC],
                                        rhs=xt[:, j0 + WP + d : j0 + WP + d + n],
                                        start=False,
                                        stop=(d == 2),
                                    )
                                else:
                                    nc.tensor.matmul(
                                        ps[:, :n],
                                        lhsT=wt[:C, (3 + d) * OC : (4 + d) * OC],
                                        rhs=xt[
                                            :C,
                                            j0 + 2 * WP + d : j0 + 2 * WP + d + n,
                                        ],
                                        start=False,
                                        stop=(d == 2),
                                    )
                            nc.vector.tensor_copy(
                                ot[:, g * BLK : g * BLK + n], ps[:, :n]
                            )
                        if g == GS - 1 or blk == NBLK - 1:
                            gn = j0 + n - g0
                            nc.sync.dma_start(
                                o_im[:, g0 : g0 + gn], ot[:, :gn]
                            )

            if repeat == 1:
                conv_pass()
            else:
                with tc.For_i(0, repeat, 1):
                    conv_pass()

    nc.compile()
    _cache[key] = nc
    return nc


def run_on_device(nc, in_maps):
    """Single-exec jitted runner with device-resident inputs; returns a
    callable for repeated timing plus the output fetcher."""
    from jax.sharding import Mesh, NamedSharding, PartitionSpec
    from jax.experimental.shard_map import shard_map
    import jax

    from concourse.bass2jax import (
        _bass_exec_p,
        install_neuronx_cc_hook,
        partition_id_tensor,
    )

    install_neuronx_cc_hook()

    partition_name = nc.partition_id_tensor.name if nc.partition_id_tensor else None
    in_names, out_names, out_avals = [], [], []
    for alloc in nc.m.functions[0].allocations:
        if not isinstance(alloc, mybir.MemoryLocationSet):
            continue
        name = alloc.memorylocations[0].name
        if alloc.kind == "ExternalInput":
            if name != partition_name:
                in_names.append(name)
        elif alloc.kind == "ExternalOutput":
            out_names.append(name)
            out_avals.append(
                jax.core.ShapedArray(
                    tuple(alloc.tensor_shape), mybir.dt.np(alloc.dtype)
                )
            )
    n_params = len(in_names)
    all_in_names = list(in_names) + list(out_names)
    if partition_name is not None:
        all_in_names.append(partition_name)
    all_in_names = tuple(all_in_names)

    def body(*args):
        operands = list(args)
        if partition_name is not None:
            operands.append(partition_id_tensor())
        return tuple(
            _bass_exec_p.bind(
                *operands,
                out_avals=tuple(out_avals),
                in_names=all_in_names,
                out_names=tuple(out_names),
                lowering_input_output_aliases=(),
                sim_require_finite=True,
                sim_require_nnan=True,
                nc=nc,
            )
        )

    n_cores = len(in_maps)
    devices = jax.devices()[:n_cores]
    mesh = Mesh(np.asarray(devices), ("core",))
    nspecs = n_params + len(out_names)
    sharded = jax.jit(
        shard_map(
            body,
            mesh=mesh,
            in_specs=(PartitionSpec("core"),) * nspecs,
            out_specs=(PartitionSpec("core"),) * len(out_names),
            check_rep=False,
        )
    )
    concat_in = [
        np.concatenate([np.asarray(in_maps[c][nm]) for c in range(n_cores)], axis=0)
        for nm in in_names
    ]
    concat_zeros = [
        np.zeros((n_cores * a.shape[0], *a.shape[1:]), a.dtype) for a in out_avals
    ]
    sharding = NamedSharding(mesh, PartitionSpec("core"))
    dev_in = [jax.device_put(a, sharding) for a in concat_in]
    dev_zeros = [jax.device_put(a, sharding) for a in concat_zeros]

    def run():
        return sharded(*dev_in, *dev_zeros)

    return run, out_names, out_avals


def _prep_inputs(x, weight):
    """Host-side shard + layout prep. Returns per-core input maps."""
    import ml_dtypes

    variant = VARIANT
    if variant.endswith("fp16"):
        host_dt = np.float16
    elif variant.endswith("bf16") or variant in ("mm_only", "wsplit9_ldw"):
        host_dt = ml_dtypes.bfloat16
    else:
        host_dt = np.float32

    xp = np.zeros((B, C, HP, WP), dtype=np.float32)
    xp[:, :, 1 : H + 1, 1 : W + 1] = x
    flat = xp.reshape(B, C, XFLAT).astype(host_dt)
    if variant == "rt2i_fp16":
        # Two images per [2C, XBUF] block: top = img 2p, bottom = img 2p+1.
        xprep = np.zeros((B // 2, 2 * C, XBUF), dtype=host_dt)
        xprep[:, :C, :XFLAT] = flat[0::2]
        xprep[:, C:, :XFLAT] = flat[1::2]
    else:
        xprep = np.zeros((B, 2 * C, XBUF), dtype=host_dt)
        xprep[:, :C, :XFLAT] = flat
        if variant in ("wsplit9_bf16", "mm_only", "wsplit9_ldw"):
            xprep[:, C:, :XFLAT] = flat
        else:
            xprep[:, C:, : XFLAT - WP] = flat[:, :, WP:]

    w4 = weight[0]  # (out_c, in_c, kh, kw)
    if variant in ("wsplit9_bf16", "mm_only", "wsplit9_ldw"):
        # lhsT slot t: rows 0-63 = bf16(w[tap t]), rows 64-127 = bf16 of the
        # residual -> contraction over both halves gives ~fp32 weight
        # precision at bf16 matmul rate.
        w_hi = w4.astype(ml_dtypes.bfloat16)
        w_lo = (w4.astype(np.float32) - w_hi.astype(np.float32)).astype(
            ml_dtypes.bfloat16
        )
        wp = np.zeros((2 * C, KH * KW, OC), dtype=host_dt)
        for t in range(KH * KW):
            kh, kw = divmod(t, KW)
            wp[:C, t] = w_hi[:, :, kh, kw].T
            wp[C:, t] = w_lo[:, :, kh, kw].T
        w_prep = np.ascontiguousarray(wp.reshape(2 * C, KH * KW * OC))
    elif variant in ("rt9_fp16", "rt2i_fp16"):
        # Row-tiled: slot t = tap (dh,dw), same [in_c, out_c] weights on both
        # partition halves (T0 reads rows 0-63, T8 rows 64-127).
        wp = np.zeros((2 * C, KH * KW, OC), dtype=host_dt)
        for t in range(KH * KW):
            kh, kw = divmod(t, KW)
            wp[:C, t] = w4[:, :, kh, kw].T.astype(host_dt)
            wp[C:, t] = wp[:C, t]
        w_prep = np.ascontiguousarray(wp.reshape(2 * C, KH * KW * OC))
    elif variant in ("pack5_fp16", "pack5r_fp16"):
        # Slots 0-2: [w(0,d); w(1,d)]. Slot 3: [w(2,0); w(2,1)] (used with
        # the on-device [img; img<<1] copy). Slot 4: [0; w(2,2)] (used with
        # the row-shifted copy at offset WP+2).
        wp = np.zeros((2 * C, 5, OC), dtype=host_dt)
        for d in range(KW):
            wp[:C, d] = w4[:, :, 0, d].T.astype(host_dt)
            wp[C:, d] = w4[:, :, 1, d].T.astype(host_dt)
        wp[:C, 3] = w4[:, :, 2, 0].T.astype(host_dt)
        wp[C:, 3] = w4[:, :, 2, 1].T.astype(host_dt)
        wp[C:, 4] = w4[:, :, 2, 2].T.astype(host_dt)
        w_prep = np.ascontiguousarray(wp.reshape(2 * C, 5 * OC))
    else:
        # lhsT slots: pairs d=0..2 pack taps (0,d) [rows 0-63] + (1,d)
        # [rows 64-127]. Singles 3+d hold tap (2,d): in rows 0-63 for the
        # K=64 variant, in rows 64-127 (zero top, used with the +WP-shifted
        # copy at offset WP+d) for the all-K=128 variant.
        wp = np.zeros((2 * C, 6, OC), dtype=host_dt)
        for d in range(KW):
            wp[:C, d] = w4[:, :, 0, d].T.astype(host_dt)
            wp[C:, d] = w4[:, :, 1, d].T.astype(host_dt)
            if variant.startswith("pack6k128"):
                wp[C:, 3 + d] = w4[:, :, 2, d].T.astype(host_dt)
            else:
                wp[:C, 3 + d] = w4[:, :, 2, d].T.astype(host_dt)
        w_prep = np.ascontiguousarray(wp.reshape(2 * C, 6 * OC))
    xpc = xprep.shape[0] // NCORES
    return [
        {"x": xprep[c * xpc : (c + 1) * xpc], "w": w_prep} for c in range(NCORES)
    ]


def kernel(x, weight):
    x = np.asarray(x, dtype=np.float32)
    weight = np.asarray(weight, dtype=np.float32)
    nc = _build()
    in_maps = _prep_inputs(x, weight)
    res = run_bass_kernel_spmd(nc, in_maps, list(range(NCORES)))
    out = np.concatenate([res.results[c]["out"] for c in range(NCORES)], axis=0)
    return np.ascontiguousarray(out[:, :, :, :W].astype(np.float32))

